# revision 3
# baseline (speedup 1.0000x reference)
"""Sparse-attention (graph-modulated MHA) Bass kernel for Trainium2.

Strategy: data-parallel over batch (8 batches -> 8 NeuronCores). Each core:
  - QKV projections as TF32 (float32r) matmuls producing transposed layouts
  - per-head scores computed transposed: sT[k_pos, q]; graph block multiplied
    on raw fp32 psum scores; key mask folded into the exp bias (per-partition)
  - softmax without max-subtraction (scores bounded); denominator L obtained
    as an extra ones-column in the attention*V matmul
  - normalization by 1/L broadcast across partitions via a DRAM round-trip DMA
  - merge projection emitted transposed; host transposes back
"""
import sys

sys.path.insert(0, "/opt/trn_rl_repo")

import numpy as np

import concourse.bass as bass
import concourse.mybir as mybir
import concourse.tile as tile
from concourse import bacc, bass_utils
from concourse.bass import ds, ts

B, S, D, H, DK = 8, 512, 1024, 16, 64
GN = 100
P = 128
NKT = S // P      # 4 key-position chunks of 128
NDT = D // P      # 8 hidden chunks of 128
NPAIR = H // 2    # 8 head pairs (2 heads share a 128-partition tile)
EH = DK + 1       # head slot width in vha (64 v-cols + 1 ones col)
F32 = mybir.dt.float32
F32R = mybir.dt.float32r
FT = mybir.ActivationFunctionType
ALU = mybir.AluOpType

_CACHE: dict = {}


def _round_tf32(x: np.ndarray) -> np.ndarray:
    """Round-to-nearest-even fp32 -> tf32 (10-bit mantissa), kept in fp32 bits."""
    b = np.ascontiguousarray(x, dtype=np.float32).view(np.uint32)
    lsb = (b >> np.uint32(13)) & np.uint32(1)
    r = b + np.uint32(0x0FFF) + lsb
    return (r & np.uint32(0xFFFFE000)).view(np.float32)


def _build_module():
    nc = bacc.Bacc("TRN2", target_bir_lowering=False, debug=False)
    dram = {}
    for nm in ("qinT", "kinT", "vinT"):
        dram[nm] = nc.dram_tensor(nm, [D, S], F32R, kind="ExternalInput").ap()
    for nm in ("wqT", "wkT", "wvT", "wmT"):
        dram[nm] = nc.dram_tensor(nm, [D, D], F32R, kind="ExternalInput").ap()
    for nm in ("bq", "bk", "bm"):
        dram[nm] = nc.dram_tensor(nm, [P, NDT], F32, kind="ExternalInput").ap()
    dram["bv"] = nc.dram_tensor("bv", [1, D], F32, kind="ExternalInput").ap()
    dram["maskb"] = nc.dram_tensor("maskb", [P, NKT], F32, kind="ExternalInput").ap()
    dram["gT"] = nc.dram_tensor("gT", [GN, GN], F32, kind="ExternalInput").ap()
    outT = nc.dram_tensor("outT", [D, S], F32, kind="ExternalOutput").ap()

    with tile.TileContext(nc) as tc:
        with (
            tc.tile_pool(name="wpool", bufs=16) as wpool,
            tc.tile_pool(name="xpool", bufs=10) as xpool,
            tc.tile_pool(name="qkpool", bufs=16) as qkpool,
            tc.tile_pool(name="vpool", bufs=4) as vpool,
            tc.tile_pool(name="ptpool", bufs=12) as ptpool,
            tc.tile_pool(name="opool", bufs=8) as opool,
            tc.tile_pool(name="outpool", bufs=3) as outpool,
            tc.tile_pool(name="cpool", bufs=1) as cpool,
            tc.tile_pool(name="rlpool", bufs=4) as rlpool,
            tc.tile_pool(name="rlbpool", bufs=4) as rlbpool,
            tc.tile_pool(name="drampool", bufs=4, space="DRAM") as drampool,
            tc.tile_pool(name="ppsum", bufs=2, space="PSUM") as ppsum,
            tc.tile_pool(name="spsum", bufs=4, space="PSUM") as spsum,
            tc.tile_pool(name="apsum", bufs=2, space="PSUM") as apsum,
        ):
            # ---- constants ----
            bqt = cpool.tile([P, NDT], F32, tag="bqt")
            nc.sync.dma_start(bqt[:], dram["bq"])
            bkt = cpool.tile([P, NDT], F32, tag="bkt")
            nc.sync.dma_start(bkt[:], dram["bk"])
            bmt = cpool.tile([P, NDT], F32, tag="bmt")
            nc.sync.dma_start(bmt[:], dram["bm"])
            maskb = cpool.tile([P, NKT], F32, tag="maskb")
            nc.sync.dma_start(maskb[:], dram["maskb"])
            gt = cpool.tile([P, GN], F32, tag="gt")
            nc.sync.dma_start(gt[0:GN, :], dram["gT"])
            bvb = cpool.tile([P, D], F32, tag="bvb")
            nc.sync.dma_start(bvb[:], dram["bv"].to_broadcast((P, D)))

            def load_chunks(name, width):
                tiles = []
                src = dram[name].rearrange("(t p) f -> t p f", p=P)
                pool = wpool if width == D else xpool
                for k_i in range(NDT):
                    t_ = pool.tile([P, width], F32R, tag="w" if width == D else "x")
                    nc.sync.dma_start(t_[:], src[k_i])
                    tiles.append(t_)
                return tiles

            # ---- Q and K projections (transposed outputs, f32r + bias) ----
            qT, kT = [], []
            for wname, xname, btile, dst in (
                ("wqT", "qinT", bqt, qT),
                ("wkT", "kinT", bkt, kT),
            ):
                wt = load_chunks(wname, D)
                xt = load_chunks(xname, S)
                for m in range(NDT):
                    ps = ppsum.tile([P, S], F32, tag="pp")
                    for k_i in range(NDT):
                        nc.tensor.matmul(
                            ps[:], wt[k_i][:, ts(m, P)], xt[k_i][:],
                            start=(k_i == 0), stop=(k_i == NDT - 1),
                        )
                    t_ = qkpool.tile([P, S], F32R, tag="qk")
                    nc.vector.tensor_scalar(t_[:], ps[:], btile[:, m : m + 1], None, ALU.add)
                    dst.append(t_)

            # ---- V projection (natural layout, packed into vha with ones col) ----
            wt = load_chunks("wvT", D)
            vt = load_chunks("vinT", S)
            vha = [vpool.tile([P, H * EH], F32R, tag="vha", name=f"vha{i}") for i in range(NKT)]
            for st in range(NKT):
                v3 = vha[st].rearrange("p (h e) -> p h e", e=EH)
                for half in range(2):
                    ps = ppsum.tile([P, S], F32, tag="pp")
                    for k_i in range(NDT):
                        nc.tensor.matmul(
                            ps[:], vt[k_i][:, ts(st, P)], wt[k_i][:, ts(half, 512)],
                            start=(k_i == 0), stop=(k_i == NDT - 1),
                        )
                    dst3 = v3[:, half * 8 : half * 8 + 8, 0:DK]
                    src3 = ps[:].rearrange("p (h d) -> p h d", d=DK)
                    bv3 = bvb[:, ts(half, 512)].rearrange("p (h d) -> p h d", d=DK)
                    nc.vector.tensor_tensor(dst3, src3, bv3, ALU.add)
                nc.vector.memset(v3[:, :, DK : DK + 1].bitcast(F32), 1.0)

            # merge weights: loaded during attention phase
            wmt = load_chunks("wmT", D)

            # ---- attention (per head pair), software-pipelined ----
            oT = [opool.tile([P, S], F32R, tag="o", name=f"oT{i}") for i in range(NPAIR)]

            def emit_scores(t):
                tiles = [[None] * NKT for _ in range(2)]
                for kc in range(NKT):
                    for x in range(2):
                        sps = spsum.tile([P, S], F32, tag="sp")
                        nc.tensor.matmul(
                            sps[:],
                            kT[t][x * DK : (x + 1) * DK, ts(kc, P)],
                            qT[t][x * DK : (x + 1) * DK, :],
                            start=True, stop=True,
                        )
                        if kc == 0:
                            nc.vector.tensor_tensor(
                                sps[0:GN, 0:GN], sps[0:GN, 0:GN], gt[0:GN, 0:GN], ALU.mult
                            )
                        pt = ptpool.tile([P, S], F32R, tag="pt")
                        nc.scalar.activation(
                            pt[:], sps[:], FT.Exp,
                            bias=maskb[:, kc : kc + 1], scale=0.125,
                        )
                        tiles[x][kc] = pt
                return tiles

            def emit_av(t, ptiles):
                for x in range(2):
                    h = 2 * t + x
                    ops = apsum.tile([EH, S], F32, tag="ap")
                    for kc in range(NKT):
                        nc.tensor.matmul(
                            ops[:], vha[kc][:, ds(h * EH, EH)], ptiles[x][kc][:],
                            start=(kc == 0), stop=(kc == NKT - 1),
                        )
                    rl = rlpool.tile([1, S], F32, tag="rl")
                    nc.vector.reciprocal(rl[:], ops[DK : DK + 1, :])
                    rld = drampool.tile([1, S], F32)
                    nc.sync.dma_start(rld[:], rl[:])
                    rlb = rlbpool.tile([DK, S], F32, tag="rlb")
                    nc.sync.dma_start(rlb[:], rld[0:1, :].to_broadcast((DK, S)))
                    nc.vector.tensor_tensor(
                        oT[t][x * DK : (x + 1) * DK, :], ops[0:DK, :], rlb[:], ALU.mult
                    )

            prev = emit_scores(0)
            for t in range(NPAIR):
                nxt = emit_scores(t + 1) if t + 1 < NPAIR else None
                emit_av(t, prev)
                prev = nxt

            # ---- merge projection (transposed output) ----
            out_view = outT.rearrange("(t p) f -> t p f", p=P)
            for m in range(NDT):
                ps = ppsum.tile([P, S], F32, tag="pp")
                for k_i in range(NDT):
                    nc.tensor.matmul(
                        ps[:], wmt[k_i][:, ts(m, P)], oT[k_i][:],
                        start=(k_i == 0), stop=(k_i == NDT - 1),
                    )
                ot = outpool.tile([P, S], F32, tag="out")
                nc.vector.tensor_scalar(ot[:], ps[:], bmt[:, m : m + 1], None, ALU.add)
                nc.sync.dma_start(out_view[m], ot[:])

    nc.compile()
    return nc


def _get_module():
    if "nc" not in _CACHE:
        _CACHE["nc"] = _build_module()
    return _CACHE["nc"]


def kernel(q, k, v, mask, graph, Wv, bv, Wk, bk, Wq, bq, Wm, bm, _trace=False):
    nc = _get_module()
    q = np.asarray(q, np.float32)
    k = np.asarray(k, np.float32)
    v = np.asarray(v, np.float32)
    mask = np.asarray(mask)
    graph = np.asarray(graph, np.float32)

    shared = {
        "wqT": _round_tf32(np.asarray(Wq, np.float32).T),
        "wkT": _round_tf32(np.asarray(Wk, np.float32).T),
        "wvT": _round_tf32(np.asarray(Wv, np.float32).T),
        "wmT": _round_tf32(np.asarray(Wm, np.float32).T),
        "bq": np.ascontiguousarray(np.asarray(bq, np.float32).reshape(NDT, P).T),
        "bk": np.ascontiguousarray(np.asarray(bk, np.float32).reshape(NDT, P).T),
        "bm": np.ascontiguousarray(np.asarray(bm, np.float32).reshape(NDT, P).T),
        "bv": np.asarray(bv, np.float32).reshape(1, D),
    }
    eye = np.eye(GN, dtype=np.float32)
    in_maps = []
    for b in range(B):
        mb = np.where(mask[b, 0, 0], np.float32(-1e9), np.float32(0.0)).astype(np.float32)
        in_maps.append(
            dict(
                shared,
                qinT=_round_tf32(q[b].T),
                kinT=_round_tf32(k[b].T),
                vinT=_round_tf32(v[b].T),
                maskb=np.ascontiguousarray(mb.reshape(NKT, P).T),
                gT=np.ascontiguousarray((graph[b] + eye).T),
            )
        )

    res = bass_utils.run_bass_kernel_spmd(
        nc, in_maps, core_ids=list(range(B)), trace=_trace
    )
    out = np.stack([r["outT"].T for r in res.results]).astype(np.float32)
    if _trace:
        kernel._last_results = res
    return out


# revision 6
# speedup vs baseline: 1.4079x; 1.4079x over previous
"""Sparse-attention (graph-modulated MHA) Bass kernel for Trainium2.

Strategy: data-parallel over batch (8 batches -> 8 NeuronCores). Each core:
  - QKV projections as bf16 matmuls producing transposed layouts (fp32 psum)
  - per-head scores computed transposed: sT[k_pos, q]; the two heads of a
    pair share one [128, 1024] psum tile (q columns side by side) so one exp
    instruction covers both; graph block multiplied on raw fp32 psum scores;
    key mask folded into the exp bias (per-partition)
  - softmax without max-subtraction (scores bounded); denominator L obtained
    as an extra ones-column in the attention*V matmul
  - all 16 reciprocals batched into one DVE op; 1/L broadcast across
    partitions via a DRAM round-trip DMA; unnormalized head outputs staged in
    fp32 and normalized+cast to bf16 just before the merge
  - merge projection emitted transposed (fp32 out); host transposes back
"""
import sys

sys.path.insert(0, "/opt/trn_rl_repo")

import ml_dtypes
import numpy as np

import concourse.bass as bass
import concourse.mybir as mybir
import concourse.tile as tile
from concourse import bacc, bass_utils
from concourse.bass import ds, ts

B, S, D, H, DK = 8, 512, 1024, 16, 64
GN = 100
P = 128
NKT = S // P      # 4 key-position chunks of 128
NDT = D // P      # 8 hidden chunks of 128
NPAIR = H // 2    # 8 head pairs (2 heads share a 128-partition tile)
EH = DK + 1       # head slot width in vha (64 v-cols + 1 ones col)
F32 = mybir.dt.float32
BF16 = mybir.dt.bfloat16
FT = mybir.ActivationFunctionType
ALU = mybir.AluOpType

_CACHE: dict = {}


def _build_module():
    nc = bacc.Bacc("TRN2", target_bir_lowering=False, debug=False)
    dram = {}
    for nm in ("qinT", "kinT", "vinT"):
        dram[nm] = nc.dram_tensor(nm, [D, S], BF16, kind="ExternalInput").ap()
    for nm in ("wqT", "wkT", "wvT", "wmT"):
        dram[nm] = nc.dram_tensor(nm, [D, D], BF16, kind="ExternalInput").ap()
    for nm in ("bq", "bk", "bm"):
        dram[nm] = nc.dram_tensor(nm, [P, NDT], F32, kind="ExternalInput").ap()
    dram["bv"] = nc.dram_tensor("bv", [1, D], F32, kind="ExternalInput").ap()
    dram["maskb"] = nc.dram_tensor("maskb", [P, NKT], F32, kind="ExternalInput").ap()
    dram["gT"] = nc.dram_tensor("gT", [GN, GN], F32, kind="ExternalInput").ap()
    outT = nc.dram_tensor("outT", [D, S], F32, kind="ExternalOutput").ap()

    with tile.TileContext(nc) as tc:
        with (
            tc.tile_pool(name="wpool", bufs=16) as wpool,
            tc.tile_pool(name="xpool", bufs=10) as xpool,
            tc.tile_pool(name="qkpool", bufs=16) as qkpool,
            tc.tile_pool(name="vpool", bufs=4) as vpool,
            tc.tile_pool(name="ptpool", bufs=8) as ptpool,
            tc.tile_pool(name="stgpool", bufs=16) as stgpool,
            tc.tile_pool(name="opool", bufs=8) as opool,
            tc.tile_pool(name="outpool", bufs=3) as outpool,
            tc.tile_pool(name="cpool", bufs=1) as cpool,
            tc.tile_pool(name="rlpool", bufs=2) as rlpool,
            tc.tile_pool(name="rlbpool", bufs=4) as rlbpool,
            tc.tile_pool(name="drampool", bufs=2, space="DRAM") as drampool,
            tc.tile_pool(name="ppsum", bufs=2, space="PSUM") as ppsum,
            tc.tile_pool(name="spsum", bufs=2, space="PSUM") as spsum,
            tc.tile_pool(name="apsum", bufs=2, space="PSUM") as apsum,
        ):
            # ---- constants ----
            bqt = cpool.tile([P, NDT], F32, tag="bqt")
            nc.sync.dma_start(bqt[:], dram["bq"])
            bkt = cpool.tile([P, NDT], F32, tag="bkt")
            nc.sync.dma_start(bkt[:], dram["bk"])
            bmt = cpool.tile([P, NDT], F32, tag="bmt")
            nc.sync.dma_start(bmt[:], dram["bm"])
            maskb = cpool.tile([P, NKT], F32, tag="maskb")
            nc.sync.dma_start(maskb[:], dram["maskb"])
            gt = cpool.tile([P, GN], F32, tag="gt")
            nc.sync.dma_start(gt[0:GN, :], dram["gT"])
            bvb = cpool.tile([P, D], F32, tag="bvb")
            nc.sync.dma_start(bvb[:], dram["bv"].to_broadcast((P, D)))
            lall = cpool.tile([H, S], F32, tag="lall")

            def load_chunks(name, width):
                tiles = []
                src = dram[name].rearrange("(t p) f -> t p f", p=P)
                pool = wpool if width == D else xpool
                for k_i in range(NDT):
                    t_ = pool.tile([P, width], BF16, tag="w" if width == D else "x")
                    nc.sync.dma_start(t_[:], src[k_i])
                    tiles.append(t_)
                return tiles

            # ---- Q and K projections (transposed outputs, bf16 + bias) ----
            qT, kT = [], []
            for wname, xname, btile, dst in (
                ("wqT", "qinT", bqt, qT),
                ("wkT", "kinT", bkt, kT),
            ):
                wt = load_chunks(wname, D)
                xt = load_chunks(xname, S)
                for m in range(NDT):
                    ps = ppsum.tile([P, S], F32, tag="pp")
                    for k_i in range(NDT):
                        nc.tensor.matmul(
                            ps[:], wt[k_i][:, ts(m, P)], xt[k_i][:],
                            start=(k_i == 0), stop=(k_i == NDT - 1),
                        )
                    t_ = qkpool.tile([P, S], BF16, tag="qk")
                    nc.vector.tensor_scalar(t_[:], ps[:], btile[:, m : m + 1], None, ALU.add)
                    dst.append(t_)

            # ---- V projection (natural layout, packed into vha with ones col) ----
            wt = load_chunks("wvT", D)
            vt = load_chunks("vinT", S)
            vha = [vpool.tile([P, H * EH], BF16, tag="vha", name=f"vha{i}") for i in range(NKT)]
            for st in range(NKT):
                v3 = vha[st].rearrange("p (h e) -> p h e", e=EH)
                for half in range(2):
                    ps = ppsum.tile([P, S], F32, tag="pp")
                    for k_i in range(NDT):
                        nc.tensor.matmul(
                            ps[:], vt[k_i][:, ts(st, P)], wt[k_i][:, ts(half, 512)],
                            start=(k_i == 0), stop=(k_i == NDT - 1),
                        )
                    dst3 = v3[:, half * 8 : half * 8 + 8, 0:DK]
                    src3 = ps[:].rearrange("p (h d) -> p h d", d=DK)
                    bv3 = bvb[:, ts(half, 512)].rearrange("p (h d) -> p h d", d=DK)
                    nc.vector.tensor_tensor(dst3, src3, bv3, ALU.add)
                nc.vector.memset(v3[:, :, DK : DK + 1], 1.0)

            # merge weights: loaded during attention phase
            wmt = load_chunks("wmT", D)

            # ---- attention (per head pair), software-pipelined ----
            # oT holds normalized bf16 outputs; stg holds unnormalized fp32
            oT = [opool.tile([P, S], BF16, tag="o", name=f"oT{i}") for i in range(NPAIR)]
            stg = [
                stgpool.tile([DK, S], F32, tag="stg", name=f"stg{i}") for i in range(H)
            ]
            ldram = drampool.tile([H, S], F32, tag="ldram")

            def emit_scores(t):
                """Both heads of pair t share one [128, 2*S] psum tile per k-chunk."""
                tiles = [None] * NKT
                for kc in range(NKT):
                    sps = spsum.tile([P, 2 * S], F32, tag="sp")
                    for x in range(2):
                        nc.tensor.matmul(
                            sps[:, ts(x, S)],
                            kT[t][x * DK : (x + 1) * DK, ts(kc, P)],
                            qT[t][x * DK : (x + 1) * DK, :],
                            start=True, stop=True,
                        )
                        if kc == 0:
                            nc.vector.tensor_tensor(
                                sps[0:GN, x * S : x * S + GN],
                                sps[0:GN, x * S : x * S + GN],
                                gt[0:GN, :], ALU.mult,
                            )
                    pt = ptpool.tile([P, 2 * S], BF16, tag="pt")
                    nc.scalar.activation(
                        pt[:], sps[:], FT.Exp,
                        bias=maskb[:, kc : kc + 1], scale=0.125,
                    )
                    tiles[kc] = pt
                return tiles

            def emit_av(t, ptiles):
                for x in range(2):
                    h = 2 * t + x
                    ops = apsum.tile([EH, S], F32, tag="ap")
                    for kc in range(NKT):
                        nc.tensor.matmul(
                            ops[:], vha[kc][:, ds(h * EH, EH)],
                            ptiles[kc][:, ts(x, S)],
                            start=(kc == 0), stop=(kc == NKT - 1),
                        )
                    lrow = rlbpool.tile([1, S], F32, tag="lrow")
                    nc.vector.tensor_copy(lrow[:], ops[DK : DK + 1, :])
                    nc.sync.dma_start(ldram[h : h + 1, :], lrow[:])
                    nc.vector.tensor_copy(stg[h][:], ops[0:DK, :])

            prev = emit_scores(0)
            for t in range(NPAIR):
                nxt = emit_scores(t + 1) if t + 1 < NPAIR else None
                emit_av(t, prev)
                prev = nxt

            # ---- batched softmax denominators -> normalize into oT ----
            nc.sync.dma_start(lall[:], ldram[:])
            rla = rlpool.tile([H, S], F32, tag="rla")
            nc.vector.reciprocal(rla[:], lall[:])
            rld = drampool.tile([H, S], F32, tag="rld")
            nc.sync.dma_start(rld[:], rla[:])
            for h in range(H):
                rlb = rlbpool.tile([DK, S], F32, tag="rlb")
                nc.sync.dma_start(rlb[:], rld[h : h + 1, :].to_broadcast((DK, S)))
                t, x = divmod(h, 2)
                nc.vector.tensor_tensor(
                    oT[t][x * DK : (x + 1) * DK, :], stg[h][:], rlb[:], ALU.mult
                )

            # ---- merge projection (transposed output) ----
            out_view = outT.rearrange("(t p) f -> t p f", p=P)
            for m in range(NDT):
                ps = ppsum.tile([P, S], F32, tag="pp")
                for k_i in range(NDT):
                    nc.tensor.matmul(
                        ps[:], wmt[k_i][:, ts(m, P)], oT[k_i][:],
                        start=(k_i == 0), stop=(k_i == NDT - 1),
                    )
                ot = outpool.tile([P, S], F32, tag="out")
                nc.vector.tensor_scalar(ot[:], ps[:], bmt[:, m : m + 1], None, ALU.add)
                nc.sync.dma_start(out_view[m], ot[:])

    nc.compile()
    return nc


def _get_module():
    if "nc" not in _CACHE:
        _CACHE["nc"] = _build_module()
    return _CACHE["nc"]


def _bf16(x: np.ndarray) -> np.ndarray:
    return np.ascontiguousarray(x, dtype=np.float32).astype(ml_dtypes.bfloat16)


def kernel(q, k, v, mask, graph, Wv, bv, Wk, bk, Wq, bq, Wm, bm, _trace=False):
    nc = _get_module()
    q = np.asarray(q, np.float32)
    k = np.asarray(k, np.float32)
    v = np.asarray(v, np.float32)
    mask = np.asarray(mask)
    graph = np.asarray(graph, np.float32)

    shared = {
        "wqT": _bf16(np.asarray(Wq, np.float32).T),
        "wkT": _bf16(np.asarray(Wk, np.float32).T),
        "wvT": _bf16(np.asarray(Wv, np.float32).T),
        "wmT": _bf16(np.asarray(Wm, np.float32).T),
        "bq": np.ascontiguousarray(np.asarray(bq, np.float32).reshape(NDT, P).T),
        "bk": np.ascontiguousarray(np.asarray(bk, np.float32).reshape(NDT, P).T),
        "bm": np.ascontiguousarray(np.asarray(bm, np.float32).reshape(NDT, P).T),
        "bv": np.asarray(bv, np.float32).reshape(1, D),
    }
    eye = np.eye(GN, dtype=np.float32)
    in_maps = []
    for b in range(B):
        mb = np.where(mask[b, 0, 0], np.float32(-1e9), np.float32(0.0)).astype(np.float32)
        in_maps.append(
            dict(
                shared,
                qinT=_bf16(q[b].T),
                kinT=_bf16(k[b].T),
                vinT=_bf16(v[b].T),
                maskb=np.ascontiguousarray(mb.reshape(NKT, P).T),
                gT=np.ascontiguousarray((graph[b] + eye).T),
            )
        )

    res = bass_utils.run_bass_kernel_spmd(
        nc, in_maps, core_ids=list(range(B)), trace=_trace
    )
    out = np.stack([r["outT"].T for r in res.results]).astype(np.float32)
    if _trace:
        kernel._last_results = res
    return out


# revision 7
# speedup vs baseline: 1.4759x; 1.0482x over previous
"""Sparse-attention (graph-modulated MHA) Bass kernel for Trainium2.

Strategy: data-parallel over batch (8 batches -> 8 NeuronCores). Per core:
  - bf16 matmuls (fp32 psum); V projection first, then Q/K projections
    interleaved per head-pair with the score matmuls so the ACT-engine exp
    work overlaps projection matmuls on the PE
  - scores computed transposed sT[k_pos, q]; the two heads of a pair share
    one [128, 1024] psum tile so one exp covers both; graph block multiplied
    on raw fp32 psum scores; key mask folded into the exp bias
  - softmax without max-subtraction; denominator L from an extra ones-column
    in the attention*V matmul; reciprocals batched (pairs 0-5 mid-loop, rest
    at the end); 1/L broadcast across partitions via DRAM round-trip DMA
  - merge projection emitted transposed (fp32 out); host transposes back
  - bulk loads on the sync DMA queue; small/late DMAs on the gpsimd queue
"""
import sys

sys.path.insert(0, "/opt/trn_rl_repo")

import ml_dtypes
import numpy as np

import concourse.bass as bass
import concourse.mybir as mybir
import concourse.tile as tile
from concourse import bacc, bass_utils
from concourse.bass import ds, ts

B, S, D, H, DK = 8, 512, 1024, 16, 64
GN = 100
P = 128
NKT = S // P      # 4 key-position chunks of 128
NDT = D // P      # 8 hidden chunks of 128
NPAIR = H // 2    # 8 head pairs (2 heads share a 128-partition tile)
EH = DK + 1       # head slot width in vha (64 v-cols + 1 ones col)
NB1 = 6           # pairs normalized in the first (mid-loop) batch
F32 = mybir.dt.float32
BF16 = mybir.dt.bfloat16
FT = mybir.ActivationFunctionType
ALU = mybir.AluOpType

_CACHE: dict = {}


def _build_module():
    nc = bacc.Bacc("TRN2", target_bir_lowering=False, debug=False)
    dram = {}
    for nm in ("qinT", "kinT", "vinT"):
        dram[nm] = nc.dram_tensor(nm, [D, S], BF16, kind="ExternalInput").ap()
    for nm in ("wqT", "wkT", "wvT", "wmT"):
        dram[nm] = nc.dram_tensor(nm, [D, D], BF16, kind="ExternalInput").ap()
    for nm in ("bq", "bk", "bm"):
        dram[nm] = nc.dram_tensor(nm, [P, NDT], F32, kind="ExternalInput").ap()
    dram["bv"] = nc.dram_tensor("bv", [1, D], F32, kind="ExternalInput").ap()
    dram["maskb"] = nc.dram_tensor("maskb", [P, NKT], F32, kind="ExternalInput").ap()
    dram["gT"] = nc.dram_tensor("gT", [GN, GN], F32, kind="ExternalInput").ap()
    outT = nc.dram_tensor("outT", [D, S], F32, kind="ExternalOutput").ap()

    with tile.TileContext(nc) as tc:
        with (
            tc.tile_pool(name="wpool", bufs=24) as wpool,
            tc.tile_pool(name="xpool", bufs=24) as xpool,
            tc.tile_pool(name="qkpool", bufs=16) as qkpool,
            tc.tile_pool(name="vpool", bufs=4) as vpool,
            tc.tile_pool(name="ptpool", bufs=10) as ptpool,
            tc.tile_pool(name="stgpool", bufs=16) as stgpool,
            tc.tile_pool(name="opool", bufs=8) as opool,
            tc.tile_pool(name="outpool", bufs=3) as outpool,
            tc.tile_pool(name="cpool", bufs=1) as cpool,
            tc.tile_pool(name="rlpool", bufs=2) as rlpool,
            tc.tile_pool(name="rlbpool", bufs=6) as rlbpool,
            tc.tile_pool(name="drampool", bufs=2, space="DRAM") as drampool,
            tc.tile_pool(name="ppsum", bufs=2, space="PSUM") as ppsum,
            tc.tile_pool(name="spsum", bufs=2, space="PSUM") as spsum,
            tc.tile_pool(name="apsum", bufs=2, space="PSUM") as apsum,
        ):
            def load_chunks(name, width):
                tiles = []
                src = dram[name].rearrange("(t p) f -> t p f", p=P)
                pool = wpool if width == D else xpool
                for k_i in range(NDT):
                    t_ = pool.tile([P, width], BF16, tag="w" if width == D else "x")
                    nc.sync.dma_start(t_[:], src[k_i])
                    tiles.append(t_)
                return tiles

            # V inputs stream first (V projection runs first)
            wvt = load_chunks("wvT", D)
            vt = load_chunks("vinT", S)

            # ---- constants (gpsimd DMA queue; small) ----
            bqt = cpool.tile([P, NDT], F32, tag="bqt")
            nc.gpsimd.dma_start(bqt[:], dram["bq"])
            bkt = cpool.tile([P, NDT], F32, tag="bkt")
            nc.gpsimd.dma_start(bkt[:], dram["bk"])
            bmt = cpool.tile([P, NDT], F32, tag="bmt")
            nc.gpsimd.dma_start(bmt[:], dram["bm"])
            maskb = cpool.tile([P, NKT], F32, tag="maskb")
            nc.gpsimd.dma_start(maskb[:], dram["maskb"])
            gt = cpool.tile([P, GN], F32, tag="gt")
            nc.gpsimd.dma_start(gt[0:GN, :], dram["gT"])
            bvb = cpool.tile([P, D], F32, tag="bvb")
            nc.gpsimd.dma_start(bvb[:], dram["bv"].to_broadcast((P, D)))

            # Q/K inputs stream behind V
            wqt = load_chunks("wqT", D)
            qt = load_chunks("qinT", S)
            wkt = load_chunks("wkT", D)
            ktc = load_chunks("kinT", S)

            # ---- V projection (natural layout, packed into vha with ones col) ----
            vha = [vpool.tile([P, H * EH], BF16, tag="vha", name=f"vha{i}") for i in range(NKT)]
            for st in range(NKT):
                v3 = vha[st].rearrange("p (h e) -> p h e", e=EH)
                for half in range(2):
                    ps = ppsum.tile([P, S], F32, tag="pp")
                    for k_i in range(NDT):
                        nc.tensor.matmul(
                            ps[:], vt[k_i][:, ts(st, P)], wvt[k_i][:, ts(half, 512)],
                            start=(k_i == 0), stop=(k_i == NDT - 1),
                        )
                    dst3 = v3[:, half * 8 : half * 8 + 8, 0:DK]
                    src3 = ps[:].rearrange("p (h d) -> p h d", d=DK)
                    bv3 = bvb[:, ts(half, 512)].rearrange("p (h d) -> p h d", d=DK)
                    nc.vector.tensor_tensor(dst3, src3, bv3, ALU.add)
                nc.vector.memset(v3[:, :, DK : DK + 1], 1.0)

            # merge weights stream during the attention phase
            wmt = load_chunks("wmT", D)

            # ---- attention state ----
            oT = [opool.tile([P, S], BF16, tag="o", name=f"oT{i}") for i in range(NPAIR)]
            stg = [
                stgpool.tile([DK, S], F32, tag="stg", name=f"stg{i}") for i in range(H)
            ]
            ldram = drampool.tile([H, S], F32, tag="ldram")
            rld = drampool.tile([H, S], F32, tag="rld")
            qT, kT = [None] * NDT, [None] * NDT

            def emit_proj(wt, xt, btile, dst, m):
                ps = ppsum.tile([P, S], F32, tag="pp")
                for k_i in range(NDT):
                    nc.tensor.matmul(
                        ps[:], wt[k_i][:, ts(m, P)], xt[k_i][:],
                        start=(k_i == 0), stop=(k_i == NDT - 1),
                    )
                t_ = qkpool.tile([P, S], BF16, tag="qk")
                nc.vector.tensor_scalar(t_[:], ps[:], btile[:, m : m + 1], None, ALU.add)
                dst[m] = t_

            def emit_scores(t):
                """Both heads of pair t share one [128, 2*S] psum tile per k-chunk."""
                tiles = [None] * NKT
                for kc in range(NKT):
                    sps = spsum.tile([P, 2 * S], F32, tag="sp")
                    for x in range(2):
                        nc.tensor.matmul(
                            sps[:, ts(x, S)],
                            kT[t][x * DK : (x + 1) * DK, ts(kc, P)],
                            qT[t][x * DK : (x + 1) * DK, :],
                            start=True, stop=True,
                        )
                        if kc == 0:
                            nc.vector.tensor_tensor(
                                sps[0:GN, x * S : x * S + GN],
                                sps[0:GN, x * S : x * S + GN],
                                gt[0:GN, :], ALU.mult,
                            )
                    pt = ptpool.tile([P, 2 * S], BF16, tag="pt")
                    nc.scalar.activation(
                        pt[:], sps[:], FT.Exp,
                        bias=maskb[:, kc : kc + 1], scale=0.125,
                    )
                    tiles[kc] = pt
                return tiles

            def emit_av(t, ptiles):
                for x in range(2):
                    h = 2 * t + x
                    ops = apsum.tile([EH, S], F32, tag="ap")
                    for kc in range(NKT):
                        nc.tensor.matmul(
                            ops[:], vha[kc][:, ds(h * EH, EH)],
                            ptiles[kc][:, ts(x, S)],
                            start=(kc == 0), stop=(kc == NKT - 1),
                        )
                    lrow = rlbpool.tile([1, S], F32, tag="lrow")
                    nc.vector.tensor_copy(lrow[:], ops[DK : DK + 1, :])
                    nc.gpsimd.dma_start(ldram[h : h + 1, :], lrow[:])
                    nc.vector.tensor_copy(stg[h][:], ops[0:DK, :])

            def emit_normalize(h0, h1, tag):
                n = h1 - h0
                la = rlpool.tile([n, S], F32, tag=f"la{tag}")
                nc.gpsimd.dma_start(la[:], ldram[h0:h1, :])
                rla = rlpool.tile([n, S], F32, tag=f"rla{tag}")
                nc.vector.reciprocal(rla[:], la[:])
                nc.gpsimd.dma_start(rld[h0:h1, :], rla[:])
                for h in range(h0, h1):
                    rlb = rlbpool.tile([DK, S], F32, tag="rlb")
                    nc.gpsimd.dma_start(rlb[:], rld[h : h + 1, :].to_broadcast((DK, S)))
                    t, x = divmod(h, 2)
                    nc.vector.tensor_tensor(
                        oT[t][x * DK : (x + 1) * DK, :], stg[h][:], rlb[:], ALU.mult
                    )

            # ---- main interleaved loop ----
            prev = None
            for t in range(NPAIR):
                emit_proj(wqt, qt, bqt, qT, t)
                emit_proj(wkt, ktc, bkt, kT, t)
                cur = emit_scores(t)
                if prev is not None:
                    emit_av(t - 1, prev)
                if t - 1 == NB1 - 1:
                    emit_normalize(0, 2 * NB1, "1")
                prev = cur
            emit_av(NPAIR - 1, prev)
            emit_normalize(2 * NB1, H, "2")

            # ---- merge projection (transposed output) ----
            out_view = outT.rearrange("(t p) f -> t p f", p=P)
            for m in range(NDT):
                ps = ppsum.tile([P, S], F32, tag="pp")
                for k_i in range(NDT):
                    nc.tensor.matmul(
                        ps[:], wmt[k_i][:, ts(m, P)], oT[k_i][:],
                        start=(k_i == 0), stop=(k_i == NDT - 1),
                    )
                ot = outpool.tile([P, S], F32, tag="out")
                nc.vector.tensor_scalar(ot[:], ps[:], bmt[:, m : m + 1], None, ALU.add)
                nc.gpsimd.dma_start(out_view[m], ot[:])

    nc.compile()
    return nc


def _get_module():
    if "nc" not in _CACHE:
        _CACHE["nc"] = _build_module()
    return _CACHE["nc"]


def _bf16(x: np.ndarray) -> np.ndarray:
    return np.ascontiguousarray(x, dtype=np.float32).astype(ml_dtypes.bfloat16)


def kernel(q, k, v, mask, graph, Wv, bv, Wk, bk, Wq, bq, Wm, bm, _trace=False):
    nc = _get_module()
    q = np.asarray(q, np.float32)
    k = np.asarray(k, np.float32)
    v = np.asarray(v, np.float32)
    mask = np.asarray(mask)
    graph = np.asarray(graph, np.float32)

    shared = {
        "wqT": _bf16(np.asarray(Wq, np.float32).T),
        "wkT": _bf16(np.asarray(Wk, np.float32).T),
        "wvT": _bf16(np.asarray(Wv, np.float32).T),
        "wmT": _bf16(np.asarray(Wm, np.float32).T),
        "bq": np.ascontiguousarray(np.asarray(bq, np.float32).reshape(NDT, P).T),
        "bk": np.ascontiguousarray(np.asarray(bk, np.float32).reshape(NDT, P).T),
        "bm": np.ascontiguousarray(np.asarray(bm, np.float32).reshape(NDT, P).T),
        "bv": np.asarray(bv, np.float32).reshape(1, D),
    }
    eye = np.eye(GN, dtype=np.float32)
    in_maps = []
    for b in range(B):
        mb = np.where(mask[b, 0, 0], np.float32(-1e9), np.float32(0.0)).astype(np.float32)
        in_maps.append(
            dict(
                shared,
                qinT=_bf16(q[b].T),
                kinT=_bf16(k[b].T),
                vinT=_bf16(v[b].T),
                maskb=np.ascontiguousarray(mb.reshape(NKT, P).T),
                gT=np.ascontiguousarray((graph[b] + eye).T),
            )
        )

    res = bass_utils.run_bass_kernel_spmd(
        nc, in_maps, core_ids=list(range(B)), trace=_trace
    )
    out = np.stack([r["outT"].T for r in res.results]).astype(np.float32)
    if _trace:
        kernel._last_results = res
    return out
